# revision 31
# baseline (speedup 1.0000x reference)
"""AttentionV1 Trainium2 Bass kernel (v6).

Data-parallel over batch: 8 images -> 8 NeuronCores. Per core:
  qkv = W_qkv @ x            (1x1 conv, PE, bf16, permuted 5-block layout)
  qkv = dwconv3x3(qkv)       (q-low, k-low, v: diag-matmul on PE;
                              q-hi/k-hi block: STT on DVE)
  qf = q*f, kf = k*f         (DVE; PE-stencil blocks fused with the
                              PSUM->SBUF move)
  G  = qf @ kf^T             (PE transpose-via-identity + PE gram)
  attn = softmax(G / (nq nk^T))  (small-tensor phase)
  out = (blockdiag(attn)^T @ W_proj^T)^T @ v   (PE)

Channel blocks (output-channel permutation of W_qkv/taps):
  B0 = q[0:128], B1 = q[128:192] || k[128:192], B2 = k[0:128],
  B3 = v[0:128], B4 = v[128:192]

v6 scheduling: PE stencil MMs (v,k,q) are emitted at the head of each
tile's tail so the PE always has ready work while the DVE finishes the
B1 stencil; this keeps the PE HAM clock-gate at 8/8 (2.4 GHz) instead
of oscillating to 4/8.
"""
import sys

for _p in ("/opt/trn_rl_repo",):
    if _p not in sys.path:
        sys.path.insert(0, _p)

import numpy as np

import concourse.bass as bass
import concourse.bacc as bacc
import concourse.mybir as mybir
from concourse.tile import TileContext
from concourse.bass_utils import run_bass_kernel_spmd

F32 = mybir.dt.float32
BF16 = mybir.dt.bfloat16
AL = mybir.AluOpType
AF = mybir.ActivationFunctionType

C = 192          # channels
O = 576          # 3*C
H = 128
W = 128
N = H * W        # 16384
HEADS = 8
CH = 24          # channels per head
TR = 8           # rows per spatial tile
NT = H // TR     # 16 tiles
S = TR * W       # 1024 spatial elems per tile
PR = TR + 2      # padded rows (halo)
PW = W + 4       # padded width: cols [2,130) hold x in [0,128)
NCHUNK = S // 128  # 8 transpose chunks per tile

BLK = [128, 128, 128, 128, 64]
QK_BLOCKS = (0, 1, 2)
TAP_OFF = [(3 * (dy + 1) + (dx + 1), dy, dx)
           for dy in (-1, 0, 1) for dx in (-1, 0, 1)]


def build_nc():
    nc = bacc.Bacc()
    x_d = nc.declare_dram_parameter("x", [C, H, W], F32, isOutput=False)
    f_d = nc.declare_dram_parameter("f", [C, H, W], F32, isOutput=False)
    wq_d = nc.declare_dram_parameter("wq", [C, O], BF16, isOutput=False)
    taps_d = nc.declare_dram_parameter("taps", [O, 9], F32, isOutput=False)
    vd3_d = nc.declare_dram_parameter("vdiag3", [128, 9 * 128], BF16, isOutput=False)
    vd4_d = nc.declare_dram_parameter("vdiag4", [64, 9 * 64], BF16, isOutput=False)
    kd3_d = nc.declare_dram_parameter("kdiag3", [128, 9 * 128], BF16, isOutput=False)
    qd3_d = nc.declare_dram_parameter("qdiag3", [128, 9 * 128], BF16, isOutput=False)
    wp_d = nc.declare_dram_parameter("wp", [C, C], BF16, isOutput=False)
    temp_d = nc.declare_dram_parameter("temp", [CH, HEADS], F32, isOutput=False)
    tlo_d = nc.declare_dram_parameter("tlo", [128, 1], F32, isOutput=False)
    thi_d = nc.declare_dram_parameter("thi", [64, 1], F32, isOutput=False)
    ones_lo_d = nc.declare_dram_parameter("ones_lo", [128, 8], F32, isOutput=False)
    ones_hi_d = nc.declare_dram_parameter("ones_hi", [64, 8], F32, isOutput=False)
    mT_lo_d = nc.declare_dram_parameter("maskT_lo", [96, C], BF16, isOutput=False)
    mT_hi_d = nc.declare_dram_parameter("maskT_hi", [96, C], BF16, isOutput=False)
    mc_lo_d = nc.declare_dram_parameter("maskc_lo", [96, 8], F32, isOutput=False)
    mc_hi_d = nc.declare_dram_parameter("maskc_hi", [96, 8], F32, isOutput=False)
    idb_d = nc.declare_dram_parameter("identb", [128, 128], BF16, isOutput=False)
    idf_d = nc.declare_dram_parameter("identf", [128, 128], F32, isOutput=False)
    out_d = nc.declare_dram_parameter("out", [C, N], BF16, isOutput=True)

    with TileContext(nc) as tc:
        with (
            tc.tile_pool(name="const", bufs=1) as cpool,
            tc.tile_pool(name="vstore", bufs=1) as vpool,
            tc.tile_pool(name="xin", bufs=3) as xpool,
            tc.tile_pool(name="fin", bufs=2) as fpool,
            tc.tile_pool(name="qkv", bufs=3) as qkvpool,
            tc.tile_pool(name="st", bufs=3) as stpool,
            tc.tile_pool(name="scr", bufs=2) as scrpool,
            tc.tile_pool(name="tsb", bufs=3) as tsbpool,
            tc.tile_pool(name="fin2", bufs=1) as finpool,
            tc.tile_pool(name="outsb", bufs=3) as outpool,
            tc.tile_pool(name="mm", bufs=2, space="PSUM") as mmpsum,
            tc.tile_pool(name="vps", bufs=1, space="PSUM") as vpsum,
            tc.tile_pool(name="tps", bufs=2, space="PSUM") as tpsum,
            tc.tile_pool(name="gram", bufs=1, space="PSUM") as gpsum,
        ):
            # ---- constants ----
            wq_sb = [cpool.tile([128, O], BF16, tag="wq0", name="wq0"),
                     cpool.tile([64, O], BF16, tag="wq1", name="wq1")]
            nc.sync.dma_start(out=wq_sb[0][:], in_=wq_d[0:128, :])
            nc.sync.dma_start(out=wq_sb[1][:], in_=wq_d[128:192, :])
            taps_sb = []
            ms = 0
            for bi, psz in enumerate(BLK):
                tt = cpool.tile([psz, 9], F32, tag=f"taps{bi}", name=f"taps{bi}")
                nc.sync.dma_start(out=tt[:], in_=taps_d[ms:ms + psz, :])
                taps_sb.append(tt)
                ms += psz
            vd3 = cpool.tile([128, 9 * 128], BF16, tag="vd3", name="vd3")
            nc.gpsimd.dma_start(out=vd3[0:64, :], in_=vd3_d[0:64, :])
            nc.sync.dma_start(out=vd3[64:128, :], in_=vd3_d[64:128, :])
            vd4 = cpool.tile([64, 9 * 64], BF16, tag="vd4", name="vd4")
            nc.sync.dma_start(out=vd4[:], in_=vd4_d[:])
            kd3 = cpool.tile([128, 9 * 128], BF16, tag="kd3", name="kd3")
            nc.gpsimd.dma_start(out=kd3[0:64, :], in_=kd3_d[0:64, :])
            nc.sync.dma_start(out=kd3[64:128, :], in_=kd3_d[64:128, :])
            qd3 = cpool.tile([128, 9 * 128], BF16, tag="qd3", name="qd3")
            nc.gpsimd.dma_start(out=qd3[0:64, :], in_=qd3_d[0:64, :])
            nc.sync.dma_start(out=qd3[64:128, :], in_=qd3_d[64:128, :])
            wp_sb = [cpool.tile([96, C], BF16, tag="wp0", name="wp0"),
                     cpool.tile([96, C], BF16, tag="wp1", name="wp1")]
            nc.sync.dma_start(out=wp_sb[0][:], in_=wp_d[0:96, :])
            nc.sync.dma_start(out=wp_sb[1][:], in_=wp_d[96:192, :])
            temp_sb = cpool.tile([CH, HEADS], F32, tag="temp", name="temp")
            nc.sync.dma_start(out=temp_sb[:], in_=temp_d[:])
            tlo_sb = cpool.tile([128, 1], F32, tag="tlo", name="tlo")
            nc.sync.dma_start(out=tlo_sb[:], in_=tlo_d[:])
            thi_sb = cpool.tile([64, 1], F32, tag="thi", name="thi")
            nc.sync.dma_start(out=thi_sb[:], in_=thi_d[:])
            ones_lo = cpool.tile([128, 8], F32, tag="ones_lo", name="ones_lo")
            nc.gpsimd.dma_start(out=ones_lo[:], in_=ones_lo_d[:])
            ones_hi = cpool.tile([64, 8], F32, tag="ones_hi", name="ones_hi")
            nc.gpsimd.dma_start(out=ones_hi[:], in_=ones_hi_d[:])
            mT_lo = cpool.tile([96, C], BF16, tag="mT_lo", name="mT_lo")
            nc.gpsimd.dma_start(out=mT_lo[:], in_=mT_lo_d[:])
            mT_hi = cpool.tile([96, C], BF16, tag="mT_hi", name="mT_hi")
            nc.gpsimd.dma_start(out=mT_hi[:], in_=mT_hi_d[:])
            mc_lo = cpool.tile([96, 8], F32, tag="mc_lo", name="mc_lo")
            nc.gpsimd.dma_start(out=mc_lo[:], in_=mc_lo_d[:])
            mc_hi = cpool.tile([96, 8], F32, tag="mc_hi", name="mc_hi")
            nc.gpsimd.dma_start(out=mc_hi[:], in_=mc_hi_d[:])
            identb = cpool.tile([128, 128], BF16, tag="identb", name="identb")
            nc.sync.dma_start(out=identb[:], in_=idb_d[:])
            identf = cpool.tile([128, 128], F32, tag="identf", name="identf")
            nc.sync.dma_start(out=identf[:], in_=idf_d[:])

            v_sb = [vpool.tile([128, N], BF16, tag="v0", name="v0"),
                    vpool.tile([64, N], BF16, tag="v1", name="v1")]
            sq_sb = [cpool.tile([128, NT], F32, tag=f"sq{i}", name=f"sq{i}")
                     for i in range(3)]
            # gram accumulators packed into one PSUM bank
            g_all = gpsum.tile([128, 512], F32, tag="g", name="g")
            g_ps = [g_all[:, 0:C], g_all[0:64, 256:256 + C]]

            def emit_stencil_mms(t, sb, dest_ps, blk_idx, diag, psz, half):
                """9-tap dwconv via diag matmul for one 128/64-ch block,
                4 output rows (half of a tile)."""
                w_cols = psz
                for ti in range(9):
                    _, dy, dx = TAP_OFF[ti]
                    r_lo = 1 + dy + 4 * half
                    rhs = sb[blk_idx][:, r_lo:r_lo + 4, 2 + dx:2 + dx + W]
                    nc.tensor.matmul(
                        dest_ps, diag[:, ti * w_cols:(ti + 1) * w_cols],
                        rhs, start=(ti == 0), stop=(ti == 8))

            def emit_v_stencil_tiled(t, sb, vp3_ps, vp4_ps, half):
                """v-low as 4 diagonal 32x32 tile chains at (i,i) and
                v-hi as 2 chains at array rows 2-3 / cols 0-1 — all 6
                chains stream concurrently on distinct subarrays."""
                for ti in range(9):
                    _, dy, dx = TAP_OFF[ti]
                    r_lo = 1 + dy + 4 * half
                    for i in range(4):
                        sl = slice(32 * i, 32 * i + 32)
                        rhs = sb[3][sl, r_lo:r_lo + 4, 2 + dx:2 + dx + W]
                        lhsT = vd3[sl, ti * 128 + 32 * i:ti * 128 + 32 * i + 32]
                        nc.tensor.matmul(
                            vp3_ps[sl, :], lhsT, rhs,
                            start=(ti == 0), stop=(ti == 8),
                            tile_position=(32 * i, 32 * i),
                            skip_group_check=True)
                    for i in range(2):
                        rsl = slice(64 + 32 * i, 96 + 32 * i)
                        osl = slice(32 * i, 32 * i + 32)
                        rhs = sb[4][rsl, r_lo:r_lo + 4, 2 + dx:2 + dx + W]
                        lhsT = vd4[rsl, ti * 64 + 32 * i:ti * 64 + 32 * i + 32]
                        nc.tensor.matmul(
                            vp4_ps[osl, :], lhsT, rhs,
                            start=(ti == 0), stop=(ti == 8),
                            tile_position=(64 + 32 * i, 32 * i),
                            skip_group_check=True)

            def emit_tail(t, st1, sb, ft_a, ft_b):
                """All PE stencils (v,k,q-low) + transposes + gram for
                tile t. Emitted AFTER tile t+1's qkv matmuls. PE stencil
                MMs lead so the PE has DVE-independent work.

                PE order: vA, kA, kB, vB, qA, qB, transposes+gram.
                DVE order: vA copies, kA-TT, kB-TT, vB copies, qA-TT,
                qB-TT, square(q-low). qA reuses kA's PSUM bank (ring 2),
                so each dependency is satisfied well before the PE
                reaches the consumer."""
                st0 = stpool.tile([128, S], BF16, tag="st0", name="st0")
                st2 = stpool.tile([128, S], BF16, tag="st2", name="st2")
                csl = [slice(0, 512), slice(512, 1024)]
                cdst = [slice(t * S, t * S + 512),
                        slice(t * S + 512, t * S + 1024)]
                # --- half A: v then k ---
                vp3a = vpsum.tile([128, 512], F32, tag="vp3", name="vp3")
                vp4a = vpsum.tile([64, 512], F32, tag="vp4", name="vp4")
                emit_stencil_mms(t, sb, vp3a[:], 3, vd3, 128, 0)
                emit_stencil_mms(t, sb, vp4a[:], 4, vd4, 64, 0)
                kpa = tpsum.tile([128, 512], F32, tag="qkt", name="kp")
                emit_stencil_mms(t, sb, kpa[:], 2, kd3, 128, 0)
                nc.vector.tensor_copy(v_sb[0][:, cdst[0]], vp3a[:])
                nc.vector.tensor_copy(v_sb[1][:, cdst[0]], vp4a[:])
                # --- half B: k then v ---
                kpb = tpsum.tile([128, 512], F32, tag="qkt", name="kp")
                emit_stencil_mms(t, sb, kpb[:], 2, kd3, 128, 1)
                nc.vector.tensor_mul(st2[:, csl[0]], kpa[:], ft_a[:, csl[0]])
                vp3b = vpsum.tile([128, 512], F32, tag="vp3", name="vp3")
                vp4b = vpsum.tile([64, 512], F32, tag="vp4", name="vp4")
                emit_stencil_mms(t, sb, vp3b[:], 3, vd3, 128, 1)
                emit_stencil_mms(t, sb, vp4b[:], 4, vd4, 64, 1)
                nc.vector.tensor_mul(st2[:, csl[1]], kpb[:], ft_a[:, csl[1]])
                nc.vector.tensor_copy(v_sb[0][:, cdst[1]], vp3b[:])
                nc.vector.tensor_copy(v_sb[1][:, cdst[1]], vp4b[:])
                # --- q-low halves (reuse the two qkt PSUM banks) ---
                qpa = tpsum.tile([128, 512], F32, tag="qkt", name="qp")
                emit_stencil_mms(t, sb, qpa[:], 0, qd3, 128, 0)
                qpb = tpsum.tile([128, 512], F32, tag="qkt", name="qp")
                emit_stencil_mms(t, sb, qpb[:], 0, qd3, 128, 1)
                nc.vector.tensor_mul(st0[:, csl[0]], qpa[:], ft_a[:, csl[0]])
                nc.vector.tensor_mul(st0[:, csl[1]], qpb[:], ft_a[:, csl[1]])
                # squares: q-low on DVE (fused reduce), k-low on Scalar
                scr0 = scrpool.tile([128, S], BF16, tag="scr0", name="scr0")
                nc.scalar.activation(
                    scr0[:], st0[:], AF.Square,
                    accum_out=sq_sb[0][:, t:t + 1])
                scr2 = scrpool.tile([128, S], BF16, tag="scr2", name="scr2")
                nc.scalar.activation(
                    scr2[:], st2[:], AF.Square,
                    accum_out=sq_sb[2][:, t:t + 1])

                # --- transposes + gram ---
                sts = [st0, st1, st2]
                for j in range(NCHUNK):
                    g = t * NCHUNK + j
                    col = slice(j * 128, (j + 1) * 128)
                    # layout: [0:128]=qt-lo, [128:256]=hi (q|k merged),
                    # [256:384]=kt-lo — one LDWEIGHTS fewer per chunk
                    # (this phase is weight-load-bound, not stream-bound)
                    qkt_ps = tpsum.tile([128, 512], F32, tag="qkt", name="qkt")
                    nc.tensor.matmul(qkt_ps[:, 0:128], sts[0][:, col],
                                     identb[:], start=True, stop=True)
                    nc.tensor.matmul(qkt_ps[:, 128:256], sts[1][:, col],
                                     identb[:], start=True, stop=True)
                    nc.tensor.matmul(qkt_ps[:, 256:384], sts[2][:, col],
                                     identb[:], start=True, stop=True)
                    qkt_sb = tsbpool.tile([128, 384], BF16, tag="qkts",
                                          name="qkts")
                    nc.scalar.activation(qkt_sb[:, 0:128],
                                         qkt_ps[:, 0:128], AF.Copy)
                    nc.scalar.activation(qkt_sb[:, 128:192],
                                         qkt_ps[:, 128:192], AF.Copy)
                    nc.scalar.activation(qkt_sb[:, 192:320],
                                         qkt_ps[:, 256:384], AF.Copy)
                    nc.scalar.activation(qkt_sb[:, 320:384],
                                         qkt_ps[:, 192:256], AF.Copy)
                    # only the FIRST matmul starts the bank's lazy-zero
                    # region (it spans all 128 partitions x whole bank);
                    # the second group's first write then sees its range
                    # fully pending and overwrites, later writes accumulate.
                    nc.tensor.matmul(
                        g_ps[0], qkt_sb[:, 0:128], qkt_sb[:, C:2 * C],
                        start=(g == 0), stop=(g == NT * NCHUNK - 1),
                        skip_group_check=True)
                    nc.tensor.matmul(
                        g_ps[1], qkt_sb[:, 128:192], qkt_sb[:, C:2 * C],
                        start=False, stop=(g == NT * NCHUNK - 1),
                        skip_group_check=True)

            prev = None
            for t in range(NT):
                r0 = t * TR
                xt = [xpool.tile([128, PR * W], BF16, tag="x0", name="x0"),
                      xpool.tile([64, PR * W], BF16, tag="x1", name="x1")]
                lo = r0 - 1
                hi = r0 + TR + 1
                dlo = max(lo, 0)
                dhi = min(hi, H)
                off = dlo - lo
                for ci, (cs, cp) in enumerate(((0, 128), (128, 64))):
                    if lo < 0:
                        nc.vector.memset(xt[ci][:, 0:W], 0.0)
                    if hi > H:
                        nc.vector.memset(xt[ci][:, (PR - 1) * W:PR * W], 0.0)
                    nc.gpsimd.dma_start(
                        out=xt[ci][:, off * W:(off + dhi - dlo) * W],
                        in_=x_d[cs:cs + cp, dlo:dhi, :],
                    )
                ft_a = fpool.tile([128, S], BF16, tag="fa", name="fa")
                nc.gpsimd.dma_start(out=ft_a[:], in_=f_d[0:128, r0:r0 + TR, :])
                ft_b = fpool.tile([128, S], BF16, tag="fb", name="fb")
                nc.gpsimd.dma_start(out=ft_b[0:64, :], in_=f_d[128:192, r0:r0 + TR, :])
                nc.gpsimd.dma_start(out=ft_b[64:128, :], in_=f_d[128:192, r0:r0 + TR, :])

                # ---- qkv matmul (5 blocks, 10 halo rows) + psum->sbuf ----
                sb = []
                ms = 0
                for bi, psz in enumerate(BLK):
                    q_sb = qkvpool.tile([psz, PR * PW], BF16, tag=f"sb{bi}",
                                        name=f"sb{bi}")
                    q3 = q_sb.rearrange("p (r w) -> p r w", w=PW)
                    sb.append(q3)
                    nc.vector.memset(q3[:, :, 1:2], 0.0)
                    nc.vector.memset(q3[:, :, 130:131], 0.0)
                    for c0, csz in ((0, 4), (4, 4), (8, 2)):
                        ps = mmpsum.tile([psz, 4 * W], F32, tag="mmps",
                                         name="mmps", bufs=3)
                        nc.tensor.matmul(
                            ps[:, :csz * W],
                            wq_sb[0][:, ms:ms + psz],
                            xt[0][:, c0 * W:(c0 + csz) * W],
                            start=True, stop=False)
                        nc.tensor.matmul(
                            ps[:, :csz * W],
                            wq_sb[1][:, ms:ms + psz],
                            xt[1][:, c0 * W:(c0 + csz) * W],
                            start=False, stop=True)
                        ps3 = ps[:, :csz * W].rearrange("p (r w) -> p r w", w=W)
                        nc.scalar.activation(
                            q3[:, c0:c0 + csz, 2:2 + W], ps3, AF.Copy)
                    ms += psz

                # previous tile's tail goes after this tile's qkv matmuls
                if prev is not None:
                    emit_tail(*prev)

                # ---- q-hi/k-hi stencil on DVE (block B1 only) ----
                bi = 1
                psz = BLK[bi]
                st1 = stpool.tile([psz, S], BF16, tag="st1", name="st1")
                acc_ap = st1.rearrange("p (r w) -> p r w", w=W)
                dve_taps = [(1, -1, 0)] + [tp for tp in TAP_OFF if tp[0] != 1]
                for idx, (ti, dy, dx) in enumerate(dve_taps):
                    src = sb[bi][:, 1 + dy:1 + dy + TR, 2 + dx:2 + dx + W]
                    w_ap = taps_sb[bi][:, ti:ti + 1]
                    if idx == 0:
                        nc.vector.tensor_scalar_mul(acc_ap, src, w_ap)
                    else:
                        nc.vector.scalar_tensor_tensor(
                            acc_ap, src, w_ap, acc_ap,
                            op0=AL.mult, op1=AL.add)

                # qf/kf multiply for B1 (in place) + squares
                nc.vector.tensor_mul(st1[:], st1[:], ft_b[:])
                scr1 = scrpool.tile([128, S], BF16, tag="scr1", name="scr1")
                nc.scalar.activation(
                    scr1[:], st1[:], AF.Square,
                    accum_out=sq_sb[1][:, t:t + 1])

                prev = (t, st1, sb, ft_a, ft_b)

            emit_tail(*prev)

            # ================= final small-tensor phase =================
            kwq = [0]

            def keep_warm(lhs_tile, kcnt, mcnt):
                """Tiny matmul dependent on a just-produced small tile to
                keep the PE HAM window busy through the scalar phase."""
                ps = mmpsum.tile([128, 512], F32, tag="mmps", name="kw",
                                 bufs=3)
                ident = identf if lhs_tile.dtype == F32 else identb
                nc.tensor.matmul(ps[0:mcnt, 0:64], lhs_tile,
                                 ident[0:kcnt, 0:64], start=True, stop=True)
                kwq[0] += 1

            dmaq = [nc.sync, nc.gpsimd]

            rb = []
            for i in range(3):
                sq1 = finpool.tile([128, 1], F32, tag=f"sq1_{i}", name=f"sq1_{i}")
                nc.vector.tensor_reduce(
                    sq1[:], sq_sb[i][:], axis=mybir.AxisListType.X, op=AL.add)
                nc.vector.tensor_scalar_max(sq1[:], sq1[:], 1e-24)
                nq = finpool.tile([128, 1], F32, tag=f"nq_{i}", name=f"nq_{i}")
                nc.scalar.activation(nq[:], sq1[:], AF.Sqrt)
                r = finpool.tile([128, 1], F32, tag=f"rq_{i}", name=f"rq_{i}")
                nc.vector.reciprocal(r[:], nq[:])
                rb.append(r)
                keep_warm(r[:], 128, 1)

            G_sb = [finpool.tile([128, C], F32, tag="G0", name="G0"),
                    finpool.tile([64, C], F32, tag="G1", name="G1")]
            nc.vector.tensor_scalar_mul(G_sb[0][:], g_ps[0], rb[0][:])
            nc.vector.tensor_scalar_mul(G_sb[1][:], g_ps[1], rb[1][0:64, :])

            gt0_t = vpsum.tile([128, 512], F32, tag="vp3", name="gt0")
            gt1_t = vpsum.tile([64, 512], F32, tag="vp4", name="gt1")
            gt_ps = [gt0_t[:, 0:C], gt1_t[:, 0:C]]
            nc.tensor.matmul(gt_ps[0][:, 0:128], G_sb[0][:, 0:128], identf[:],
                             is_transpose=True, start=True, stop=True)
            nc.tensor.matmul(gt_ps[0][:, 128:192], G_sb[1][:, 0:128],
                             identf[0:64, 0:64], is_transpose=True,
                             start=True, stop=True)
            nc.tensor.matmul(gt_ps[1][:, 0:128], G_sb[0][:, 128:192], identf[:],
                             is_transpose=True, start=True, stop=True)
            nc.tensor.matmul(gt_ps[1][:, 128:192], G_sb[1][:, 128:192],
                             identf[0:64, 0:64], is_transpose=True,
                             start=True, stop=True)

            # per-partition rk * temperature vectors for gt rows (d)
            rkt_lo = finpool.tile([128, 1], F32, tag="rktl", name="rktl")
            nc.vector.tensor_mul(rkt_lo[:], rb[2][:], tlo_sb[:])
            rkh0 = finpool.tile([64, 1], F32, tag="rkh0", name="rkh0")
            nc.gpsimd.dma_start(out=rkh0[:], in_=rb[1][64:128, :])
            rkt_hi = finpool.tile([64, 1], F32, tag="rkth", name="rkth")
            nc.vector.tensor_mul(rkt_hi[:], rkh0[:], thi_sb[:])
            keep_warm(rkt_lo[:], 128, 1)

            gt_sb = [finpool.tile([128, C], F32, tag="gts0", name="gts0"),
                     finpool.tile([64, C], F32, tag="gts1", name="gts1")]
            nc.vector.tensor_scalar_mul(gt_sb[0][:], gt_ps[0], rkt_lo[:])
            nc.vector.tensor_scalar_mul(gt_sb[1][:], gt_ps[1], rkt_hi[:])
            keep_warm(gt_sb[0][:, 0:64], 128, 64)
            # ---- softmax without gathers: exp the full scaled Gt,
            # per-head denominators via block-ones matmul, division folded
            # into W_proj row-scaling, cross-head masking fused into the
            # attn-transpose copy ----
            e_lo = finpool.tile([128, C], F32, tag="elo", name="elo")
            nc.scalar.activation(e_lo[:], gt_sb[0][:], AF.Exp)
            e_hi = finpool.tile([64, C], F32, tag="ehi", name="ehi")
            nc.scalar.activation(e_hi[:], gt_sb[1][:], AF.Exp)
            # denominators: sums[h, c] = sum_{d in head h} e[d, c]
            sums_t = tpsum.tile([128, 512], F32, tag="qkt", name="sums")
            sums_ps = sums_t[0:8, 0:C]
            nc.tensor.matmul(sums_ps, ones_lo[:], e_lo[:],
                             start=True, stop=False)
            nc.tensor.matmul(sums_ps, ones_hi[:], e_hi[:],
                             start=False, stop=True)
            sums_sb = finpool.tile([8, C], F32, tag="sums", name="sums")
            nc.vector.tensor_copy(sums_sb[:], sums_ps)
            # attn^T (c on partitions) with cross-head masking in the copy
            aT_t = tpsum.tile([128, 512], F32, tag="qkt", name="aT")
            nc.tensor.matmul(aT_t[0:96, 0:128], e_lo[:, 0:96], identf[:],
                             start=True, stop=True, skip_group_check=True)
            nc.tensor.matmul(aT_t[0:96, 128:192], e_hi[:, 0:96],
                             identf[0:64, 0:64], start=True, stop=True,
                             skip_group_check=True)
            nc.tensor.matmul(aT_t[0:96, 256:384], e_lo[:, 96:192], identf[:],
                             start=True, stop=True, skip_group_check=True)
            nc.tensor.matmul(aT_t[0:96, 384:448], e_hi[:, 96:192],
                             identf[0:64, 0:64], start=True, stop=True,
                             skip_group_check=True)
            # transpose sums -> [96, 8] halves, pick own-head entry, recip
            sT_t = tpsum.tile([128, 512], F32, tag="qkt", name="sT")
            nc.tensor.matmul(sT_t[0:96, 0:8], sums_sb[:, 0:96],
                             identf[0:8, 0:8], start=True, stop=True,
                             skip_group_check=True)
            nc.tensor.matmul(sT_t[0:96, 8:16], sums_sb[:, 96:192],
                             identf[0:8, 0:8], start=True, stop=True,
                             skip_group_check=True)
            den = finpool.tile([96, 16], F32, tag="den", name="den")
            nc.vector.tensor_mul(den[:, 0:8], sT_t[0:96, 0:8], mc_lo[:])
            nc.vector.tensor_mul(den[:, 8:16], sT_t[0:96, 8:16], mc_hi[:])
            dsum = finpool.tile([96, 2], F32, tag="dsum", name="dsum")
            nc.vector.tensor_reduce(
                dsum[:, 0:1], den[:, 0:8], axis=mybir.AxisListType.X,
                op=AL.add)
            nc.vector.tensor_reduce(
                dsum[:, 1:2], den[:, 8:16], axis=mybir.AxisListType.X,
                op=AL.add)
            rsr = finpool.tile([96, 2], F32, tag="rsr", name="rsr")
            nc.vector.reciprocal(rsr[:], dsum[:])
            # scaled projection weights: wps[k] = wp[k] * (1/denominator)
            wps_sb = [finpool.tile([96, C], BF16, tag="wps0", name="wps0"),
                      finpool.tile([96, C], BF16, tag="wps1", name="wps1")]
            nc.vector.tensor_scalar_mul(wps_sb[0][:], wp_sb[0][:],
                                        rsr[:, 0:1])
            nc.vector.tensor_scalar_mul(wps_sb[1][:], wp_sb[1][:],
                                        rsr[:, 1:2])
            # masked attn^T to SBUF (bf16)
            attnT = [finpool.tile([96, C], BF16, tag="aT0", name="aT0"),
                     finpool.tile([96, C], BF16, tag="aT1", name="aT1")]
            nc.vector.tensor_mul(attnT[0][:], aT_t[0:96, 0:C], mT_lo[:])
            nc.vector.tensor_mul(attnT[1][:], aT_t[0:96, 256:256 + C],
                                 mT_hi[:])
            mt0_t = tpsum.tile([128, 512], F32, tag="qkt", name="mt0")
            mt1_t = tpsum.tile([128, 512], F32, tag="qkt", name="mt1")
            mt_ps = [mt0_t[:, 0:C], mt1_t[0:64, 0:C]]
            for mi, msl in enumerate((slice(0, 128), slice(128, 192))):
                for k in range(2):
                    nc.tensor.matmul(mt_ps[mi], attnT[k][:, msl], wps_sb[k][:],
                                     start=(k == 0), stop=(k == 1))
            # dense PE burst (~4us) to push the HAM clock-gate back to
            # 8/8 before the long out-matmul stream; results unused
            for wi in range(18):
                wps = mmpsum.tile([128, 512], F32, tag="mmps", name="wps",
                                  bufs=3)
                nc.tensor.matmul(wps[:], identb[:],
                                 v_sb[0][:, wi * 512:(wi + 1) * 512],
                                 start=True, stop=True)
            mt_sb = [finpool.tile([128, C], BF16, tag="mt_sb0", name="mt_sb0"),
                     finpool.tile([64, C], BF16, tag="mt_sb1", name="mt_sb1")]
            nc.vector.tensor_copy(mt_sb[0][:], mt_ps[0])
            nc.vector.tensor_copy(mt_sb[1][:], mt_ps[1])

            # out matmuls in groups of 3 columns-chunks per weight load:
            # same lhsT streams 3 chunks back-to-back (one LDWEIGHTS per 3
            # matmuls), 6 PSUM banks in flight (mmps x3 + qkt x2 + vp3)
            jlist = list(range(N // 512))
            gi = 0
            gsizes = [3] * 8 + [2] * 3 + [1] * 2
            while jlist:
                gs = gsizes.pop(0)
                grp = jlist[:gs]
                jlist = jlist[gs:]
                ps0 = {}
                ps1 = {}
                for gj, j in enumerate(grp):
                    ps0[j] = mmpsum.tile([128, 512], F32, tag="mmps",
                                         name="mmps", bufs=3)
                    if gj < 2:
                        ps1[j] = tpsum.tile([128, 512], F32, tag="qkt",
                                            name="ops")[0:64, :]
                    else:
                        ps1[j] = vpsum.tile([128, 512], F32, tag="vp3",
                                            name="ops")[0:64, :]
                for k in range(2):
                    for j in grp:
                        col = slice(j * 512, (j + 1) * 512)
                        nc.tensor.matmul(ps0[j][:], mt_sb[k][:, 0:128],
                                         v_sb[k][:, col],
                                         start=(k == 0), stop=(k == 1))
                for k in range(2):
                    for j in grp:
                        col = slice(j * 512, (j + 1) * 512)
                        nc.tensor.matmul(ps1[j][:], mt_sb[k][:, 128:192],
                                         v_sb[k][:, col],
                                         start=(k == 0), stop=(k == 1))
                gw = 512 * len(grp)
                gcol = slice(grp[0] * 512, grp[0] * 512 + gw)
                osb0 = outpool.tile([128, 1536], BF16, tag="osb0",
                                    name="osb0")
                osb1 = outpool.tile([64, 1536], BF16, tag="osb1",
                                    name="osb1")
                for gj, j in enumerate(grp):
                    gsl = slice(gj * 512, (gj + 1) * 512)
                    nc.scalar.activation(osb0[:, gsl], ps0[j][:], AF.Copy)
                    nc.vector.tensor_copy(osb1[0:64, gsl], ps1[j][:])
                dq = nc.sync if gi % 2 == 0 else nc.gpsimd
                dq.dma_start(out=out_d[0:128, gcol], in_=osb0[:, 0:gw])
                dq2 = nc.gpsimd if gi % 2 == 0 else nc.sync
                dq2.dma_start(out=out_d[128:192, gcol], in_=osb1[0:64, 0:gw])
                gi += 1
    nc.finalize()
    return nc


_NC_CACHE = {}


def _perm():
    return (list(range(0, 128)) + list(range(128, 192))
            + list(range(320, 384)) + list(range(192, 320))
            + list(range(384, 576)))


def _diag9(taps_blk, psz):
    d = np.zeros((psz, 9 * psz), np.float32)
    idx = np.arange(psz)
    for ti in range(9):
        d[:, ti * psz:(ti + 1) * psz][idx, idx] = taps_blk[:, ti]
    return d


def kernel(x, feature, W_qkv, W_dw, W_proj, temperature):
    import ml_dtypes
    b = x.shape[0]
    perm = _perm()
    wq_p = np.asarray(W_qkv, np.float32)[perm, :]
    wq = np.ascontiguousarray(wq_p.T).astype(ml_dtypes.bfloat16)
    taps = np.ascontiguousarray(
        np.asarray(W_dw, np.float32).reshape(O, 9)[perm, :])
    vd3 = _diag9(taps[384:512, :], 128)
    vd4 = _diag9(taps[512:576, :], 64)
    kd3 = _diag9(taps[256:384, :], 128)
    qd3 = _diag9(taps[0:128, :], 128)
    wp = np.ascontiguousarray(np.asarray(W_proj, np.float32).T).astype(
        ml_dtypes.bfloat16)
    temp = np.broadcast_to(
        np.asarray(temperature, np.float32).reshape(1, HEADS), (CH, HEADS))
    temp = np.ascontiguousarray(temp)
    tvec = np.repeat(np.asarray(temperature, np.float32).reshape(HEADS), CH)
    tlo = np.ascontiguousarray(tvec[0:128].reshape(128, 1))
    thi = np.ascontiguousarray(tvec[128:192].reshape(64, 1))
    hid = np.arange(C) // CH
    ones_bd = (hid[:, None] == np.arange(HEADS)[None, :]).astype(np.float32)
    maskT = (hid[:, None] == hid[None, :]).astype(np.float32)

    if "nc" not in _NC_CACHE:
        _NC_CACHE["nc"] = build_nc()
    nc = _NC_CACHE["nc"]

    in_maps = []
    for i in range(b):
        in_maps.append({
            "x": np.ascontiguousarray(np.asarray(x[i], np.float32)),
            "f": np.ascontiguousarray(np.asarray(feature[i], np.float32)),
            "wq": wq, "taps": taps,
            "vdiag3": vd3.astype(ml_dtypes.bfloat16),
            "vdiag4": vd4.astype(ml_dtypes.bfloat16),
            "kdiag3": kd3.astype(ml_dtypes.bfloat16),
            "qdiag3": qd3.astype(ml_dtypes.bfloat16),
            "wp": wp, "temp": temp, "tlo": tlo, "thi": thi,
            "ones_lo": np.ascontiguousarray(ones_bd[0:128]),
            "ones_hi": np.ascontiguousarray(ones_bd[128:192]),
            "maskT_lo": np.ascontiguousarray(maskT[0:96]).astype(
                ml_dtypes.bfloat16),
            "maskT_hi": np.ascontiguousarray(maskT[96:192]).astype(
                ml_dtypes.bfloat16),
            "maskc_lo": np.ascontiguousarray(ones_bd[0:96]),
            "maskc_hi": np.ascontiguousarray(ones_bd[96:192]),
            "identb": np.eye(128, dtype=np.float32).astype(ml_dtypes.bfloat16),
            "identf": np.eye(128, dtype=np.float32),
        })
    res = run_bass_kernel_spmd(nc, in_maps, list(range(b)))
    outs = [np.asarray(r["out"], np.float32).reshape(C, H, W)
            for r in res.results]
    return np.stack(outs, axis=0)


# revision 32
# speedup vs baseline: 1.0008x; 1.0008x over previous
"""AttentionV1 Trainium2 Bass kernel (v6).

Data-parallel over batch: 8 images -> 8 NeuronCores. Per core:
  qkv = W_qkv @ x            (1x1 conv, PE, bf16, permuted 5-block layout)
  qkv = dwconv3x3(qkv)       (q-low, k-low, v: diag-matmul on PE;
                              q-hi/k-hi block: STT on DVE)
  qf = q*f, kf = k*f         (DVE; PE-stencil blocks fused with the
                              PSUM->SBUF move)
  G  = qf @ kf^T             (PE transpose-via-identity + PE gram)
  attn = softmax(G / (nq nk^T))  (small-tensor phase)
  out = (blockdiag(attn)^T @ W_proj^T)^T @ v   (PE)

Channel blocks (output-channel permutation of W_qkv/taps):
  B0 = q[0:128], B1 = q[128:192] || k[128:192], B2 = k[0:128],
  B3 = v[0:128], B4 = v[128:192]

v6 scheduling: PE stencil MMs (v,k,q) are emitted at the head of each
tile's tail so the PE always has ready work while the DVE finishes the
B1 stencil; this keeps the PE HAM clock-gate at 8/8 (2.4 GHz) instead
of oscillating to 4/8.
"""
import sys

for _p in ("/opt/trn_rl_repo",):
    if _p not in sys.path:
        sys.path.insert(0, _p)

import numpy as np

import concourse.bass as bass
import concourse.bacc as bacc
import concourse.mybir as mybir
from concourse.tile import TileContext
from concourse.bass_utils import run_bass_kernel_spmd

F32 = mybir.dt.float32
BF16 = mybir.dt.bfloat16
AL = mybir.AluOpType
AF = mybir.ActivationFunctionType

C = 192          # channels
O = 576          # 3*C
H = 128
W = 128
N = H * W        # 16384
HEADS = 8
CH = 24          # channels per head
TR = 8           # rows per spatial tile
NT = H // TR     # 16 tiles
S = TR * W       # 1024 spatial elems per tile
PR = TR + 2      # padded rows (halo)
PW = W + 4       # padded width: cols [2,130) hold x in [0,128)
NCHUNK = S // 128  # 8 transpose chunks per tile

BLK = [128, 128, 128, 128, 64]
QK_BLOCKS = (0, 1, 2)
TAP_OFF = [(3 * (dy + 1) + (dx + 1), dy, dx)
           for dy in (-1, 0, 1) for dx in (-1, 0, 1)]


def build_nc():
    nc = bacc.Bacc()
    x_d = nc.declare_dram_parameter("x", [C, H, W], F32, isOutput=False)
    f_d = nc.declare_dram_parameter("f", [C, H, W], F32, isOutput=False)
    wq_d = nc.declare_dram_parameter("wq", [C, O], BF16, isOutput=False)
    taps_d = nc.declare_dram_parameter("taps", [O, 9], F32, isOutput=False)
    vd3_d = nc.declare_dram_parameter("vdiag3", [128, 9 * 128], BF16, isOutput=False)
    vd4_d = nc.declare_dram_parameter("vdiag4", [64, 9 * 64], BF16, isOutput=False)
    kd3_d = nc.declare_dram_parameter("kdiag3", [128, 9 * 128], BF16, isOutput=False)
    qd3_d = nc.declare_dram_parameter("qdiag3", [128, 9 * 128], BF16, isOutput=False)
    wp_d = nc.declare_dram_parameter("wp", [C, C], BF16, isOutput=False)
    temp_d = nc.declare_dram_parameter("temp", [CH, HEADS], F32, isOutput=False)
    tlo_d = nc.declare_dram_parameter("tlo", [128, 1], F32, isOutput=False)
    thi_d = nc.declare_dram_parameter("thi", [64, 1], F32, isOutput=False)
    ones_lo_d = nc.declare_dram_parameter("ones_lo", [128, 8], F32, isOutput=False)
    ones_hi_d = nc.declare_dram_parameter("ones_hi", [64, 8], F32, isOutput=False)
    mT_lo_d = nc.declare_dram_parameter("maskT_lo", [96, C], BF16, isOutput=False)
    mT_hi_d = nc.declare_dram_parameter("maskT_hi", [96, C], BF16, isOutput=False)
    mc_lo_d = nc.declare_dram_parameter("maskc_lo", [96, 8], F32, isOutput=False)
    mc_hi_d = nc.declare_dram_parameter("maskc_hi", [96, 8], F32, isOutput=False)
    idb_d = nc.declare_dram_parameter("identb", [128, 128], BF16, isOutput=False)
    idf_d = nc.declare_dram_parameter("identf", [128, 128], F32, isOutput=False)
    out_d = nc.declare_dram_parameter("out", [C, N], BF16, isOutput=True)

    with TileContext(nc) as tc:
        with (
            tc.tile_pool(name="const", bufs=1) as cpool,
            tc.tile_pool(name="vstore", bufs=1) as vpool,
            tc.tile_pool(name="xin", bufs=3) as xpool,
            tc.tile_pool(name="fin", bufs=2) as fpool,
            tc.tile_pool(name="qkv", bufs=3) as qkvpool,
            tc.tile_pool(name="st", bufs=3) as stpool,
            tc.tile_pool(name="scr", bufs=2) as scrpool,
            tc.tile_pool(name="tsb", bufs=3) as tsbpool,
            tc.tile_pool(name="fin2", bufs=1) as finpool,
            tc.tile_pool(name="outsb", bufs=3) as outpool,
            tc.tile_pool(name="mm", bufs=2, space="PSUM") as mmpsum,
            tc.tile_pool(name="vps", bufs=1, space="PSUM") as vpsum,
            tc.tile_pool(name="tps", bufs=2, space="PSUM") as tpsum,
            tc.tile_pool(name="gram", bufs=1, space="PSUM") as gpsum,
        ):
            # ---- constants ----
            wq_sb = [cpool.tile([128, O], BF16, tag="wq0", name="wq0"),
                     cpool.tile([64, O], BF16, tag="wq1", name="wq1")]
            nc.sync.dma_start(out=wq_sb[0][:], in_=wq_d[0:128, :])
            nc.sync.dma_start(out=wq_sb[1][:], in_=wq_d[128:192, :])
            taps_sb = []
            ms = 0
            for bi, psz in enumerate(BLK):
                tt = cpool.tile([psz, 9], F32, tag=f"taps{bi}", name=f"taps{bi}")
                nc.sync.dma_start(out=tt[:], in_=taps_d[ms:ms + psz, :])
                taps_sb.append(tt)
                ms += psz
            vd3 = cpool.tile([128, 9 * 128], BF16, tag="vd3", name="vd3")
            nc.gpsimd.dma_start(out=vd3[0:64, :], in_=vd3_d[0:64, :])
            nc.sync.dma_start(out=vd3[64:128, :], in_=vd3_d[64:128, :])
            vd4 = cpool.tile([64, 9 * 64], BF16, tag="vd4", name="vd4")
            nc.sync.dma_start(out=vd4[:], in_=vd4_d[:])
            kd3 = cpool.tile([128, 9 * 128], BF16, tag="kd3", name="kd3")
            nc.gpsimd.dma_start(out=kd3[0:64, :], in_=kd3_d[0:64, :])
            nc.sync.dma_start(out=kd3[64:128, :], in_=kd3_d[64:128, :])
            qd3 = cpool.tile([128, 9 * 128], BF16, tag="qd3", name="qd3")
            nc.gpsimd.dma_start(out=qd3[0:64, :], in_=qd3_d[0:64, :])
            nc.sync.dma_start(out=qd3[64:128, :], in_=qd3_d[64:128, :])
            wp_sb = [cpool.tile([96, C], BF16, tag="wp0", name="wp0"),
                     cpool.tile([96, C], BF16, tag="wp1", name="wp1")]
            nc.sync.dma_start(out=wp_sb[0][:], in_=wp_d[0:96, :])
            nc.sync.dma_start(out=wp_sb[1][:], in_=wp_d[96:192, :])
            temp_sb = cpool.tile([CH, HEADS], F32, tag="temp", name="temp")
            nc.sync.dma_start(out=temp_sb[:], in_=temp_d[:])
            tlo_sb = cpool.tile([128, 1], F32, tag="tlo", name="tlo")
            nc.sync.dma_start(out=tlo_sb[:], in_=tlo_d[:])
            thi_sb = cpool.tile([64, 1], F32, tag="thi", name="thi")
            nc.sync.dma_start(out=thi_sb[:], in_=thi_d[:])
            ones_lo = cpool.tile([128, 8], F32, tag="ones_lo", name="ones_lo")
            nc.gpsimd.dma_start(out=ones_lo[:], in_=ones_lo_d[:])
            ones_hi = cpool.tile([64, 8], F32, tag="ones_hi", name="ones_hi")
            nc.gpsimd.dma_start(out=ones_hi[:], in_=ones_hi_d[:])
            mT_lo = cpool.tile([96, C], BF16, tag="mT_lo", name="mT_lo")
            nc.gpsimd.dma_start(out=mT_lo[:], in_=mT_lo_d[:])
            mT_hi = cpool.tile([96, C], BF16, tag="mT_hi", name="mT_hi")
            nc.gpsimd.dma_start(out=mT_hi[:], in_=mT_hi_d[:])
            mc_lo = cpool.tile([96, 8], F32, tag="mc_lo", name="mc_lo")
            nc.gpsimd.dma_start(out=mc_lo[:], in_=mc_lo_d[:])
            mc_hi = cpool.tile([96, 8], F32, tag="mc_hi", name="mc_hi")
            nc.gpsimd.dma_start(out=mc_hi[:], in_=mc_hi_d[:])
            identb = cpool.tile([128, 128], BF16, tag="identb", name="identb")
            nc.sync.dma_start(out=identb[:], in_=idb_d[:])
            identf = cpool.tile([128, 128], F32, tag="identf", name="identf")
            nc.sync.dma_start(out=identf[:], in_=idf_d[:])

            v_sb = [vpool.tile([128, N], BF16, tag="v0", name="v0"),
                    vpool.tile([64, N], BF16, tag="v1", name="v1")]
            sq_sb = [cpool.tile([128, NT], F32, tag=f"sq{i}", name=f"sq{i}")
                     for i in range(3)]
            # gram accumulators packed into one PSUM bank
            g_all = gpsum.tile([128, 512], F32, tag="g", name="g")
            g_ps = [g_all[:, 0:C], g_all[0:64, 256:256 + C]]

            def emit_stencil_mms(t, sb, dest_ps, blk_idx, diag, psz, half):
                """9-tap dwconv via diag matmul for one 128/64-ch block,
                4 output rows (half of a tile)."""
                w_cols = psz
                for ti in range(9):
                    _, dy, dx = TAP_OFF[ti]
                    r_lo = 1 + dy + 4 * half
                    rhs = sb[blk_idx][:, r_lo:r_lo + 4, 2 + dx:2 + dx + W]
                    nc.tensor.matmul(
                        dest_ps, diag[:, ti * w_cols:(ti + 1) * w_cols],
                        rhs, start=(ti == 0), stop=(ti == 8))

            def emit_v_stencil_tiled(t, sb, vp3_ps, vp4_ps, half):
                """v-low as 4 diagonal 32x32 tile chains at (i,i) and
                v-hi as 2 chains at array rows 2-3 / cols 0-1 — all 6
                chains stream concurrently on distinct subarrays."""
                for ti in range(9):
                    _, dy, dx = TAP_OFF[ti]
                    r_lo = 1 + dy + 4 * half
                    for i in range(4):
                        sl = slice(32 * i, 32 * i + 32)
                        rhs = sb[3][sl, r_lo:r_lo + 4, 2 + dx:2 + dx + W]
                        lhsT = vd3[sl, ti * 128 + 32 * i:ti * 128 + 32 * i + 32]
                        nc.tensor.matmul(
                            vp3_ps[sl, :], lhsT, rhs,
                            start=(ti == 0), stop=(ti == 8),
                            tile_position=(32 * i, 32 * i),
                            skip_group_check=True)
                    for i in range(2):
                        rsl = slice(64 + 32 * i, 96 + 32 * i)
                        osl = slice(32 * i, 32 * i + 32)
                        rhs = sb[4][rsl, r_lo:r_lo + 4, 2 + dx:2 + dx + W]
                        lhsT = vd4[rsl, ti * 64 + 32 * i:ti * 64 + 32 * i + 32]
                        nc.tensor.matmul(
                            vp4_ps[osl, :], lhsT, rhs,
                            start=(ti == 0), stop=(ti == 8),
                            tile_position=(64 + 32 * i, 32 * i),
                            skip_group_check=True)

            def emit_tail(t, st1, sb, ft_a, ft_b):
                """All PE stencils (v,k,q-low) + transposes + gram for
                tile t. Emitted AFTER tile t+1's qkv matmuls. PE stencil
                MMs lead so the PE has DVE-independent work.

                PE order: vA, kA, kB, vB, qA, qB, transposes+gram.
                DVE order: vA copies, kA-TT, kB-TT, vB copies, qA-TT,
                qB-TT, square(q-low). qA reuses kA's PSUM bank (ring 2),
                so each dependency is satisfied well before the PE
                reaches the consumer."""
                st0 = stpool.tile([128, S], BF16, tag="st0", name="st0")
                st2 = stpool.tile([128, S], BF16, tag="st2", name="st2")
                csl = [slice(0, 512), slice(512, 1024)]
                cdst = [slice(t * S, t * S + 512),
                        slice(t * S + 512, t * S + 1024)]
                # --- half A: v then k ---
                vp3a = vpsum.tile([128, 512], F32, tag="vp3", name="vp3")
                vp4a = vpsum.tile([64, 512], F32, tag="vp4", name="vp4")
                emit_stencil_mms(t, sb, vp3a[:], 3, vd3, 128, 0)
                emit_stencil_mms(t, sb, vp4a[:], 4, vd4, 64, 0)
                kpa = tpsum.tile([128, 512], F32, tag="qkt", name="kp")
                emit_stencil_mms(t, sb, kpa[:], 2, kd3, 128, 0)
                nc.vector.tensor_copy(v_sb[0][:, cdst[0]], vp3a[:])
                nc.vector.tensor_copy(v_sb[1][:, cdst[0]], vp4a[:])
                # --- half B: k then v ---
                kpb = tpsum.tile([128, 512], F32, tag="qkt", name="kp")
                emit_stencil_mms(t, sb, kpb[:], 2, kd3, 128, 1)
                nc.vector.tensor_mul(st2[:, csl[0]], kpa[:], ft_a[:, csl[0]])
                vp3b = vpsum.tile([128, 512], F32, tag="vp3", name="vp3")
                vp4b = vpsum.tile([64, 512], F32, tag="vp4", name="vp4")
                emit_stencil_mms(t, sb, vp3b[:], 3, vd3, 128, 1)
                emit_stencil_mms(t, sb, vp4b[:], 4, vd4, 64, 1)
                nc.vector.tensor_mul(st2[:, csl[1]], kpb[:], ft_a[:, csl[1]])
                nc.vector.tensor_copy(v_sb[0][:, cdst[1]], vp3b[:])
                nc.vector.tensor_copy(v_sb[1][:, cdst[1]], vp4b[:])
                # --- q-low halves (reuse the two qkt PSUM banks) ---
                qpa = tpsum.tile([128, 512], F32, tag="qkt", name="qp")
                emit_stencil_mms(t, sb, qpa[:], 0, qd3, 128, 0)
                qpb = tpsum.tile([128, 512], F32, tag="qkt", name="qp")
                emit_stencil_mms(t, sb, qpb[:], 0, qd3, 128, 1)
                nc.vector.tensor_mul(st0[:, csl[0]], qpa[:], ft_a[:, csl[0]])
                nc.vector.tensor_mul(st0[:, csl[1]], qpb[:], ft_a[:, csl[1]])
                # squares: q-low on DVE (fused reduce), k-low on Scalar
                scr0 = scrpool.tile([128, S], BF16, tag="scr0", name="scr0")
                nc.scalar.activation(
                    scr0[:], st0[:], AF.Square,
                    accum_out=sq_sb[0][:, t:t + 1])
                scr2 = scrpool.tile([128, S], BF16, tag="scr2", name="scr2")
                nc.scalar.activation(
                    scr2[:], st2[:], AF.Square,
                    accum_out=sq_sb[2][:, t:t + 1])

                # --- transposes + gram ---
                sts = [st0, st1, st2]
                for j in range(NCHUNK):
                    g = t * NCHUNK + j
                    col = slice(j * 128, (j + 1) * 128)
                    # layout: [0:128]=qt-lo, [128:256]=hi (q|k merged),
                    # [256:384]=kt-lo — one LDWEIGHTS fewer per chunk
                    # (this phase is weight-load-bound, not stream-bound)
                    qkt_ps = tpsum.tile([128, 512], F32, tag="qkt", name="qkt")
                    nc.tensor.matmul(qkt_ps[:, 0:128], sts[0][:, col],
                                     identb[:], start=True, stop=True)
                    nc.tensor.matmul(qkt_ps[:, 128:256], sts[1][:, col],
                                     identb[:], start=True, stop=True)
                    nc.tensor.matmul(qkt_ps[:, 256:384], sts[2][:, col],
                                     identb[:], start=True, stop=True)
                    qkt_sb = tsbpool.tile([128, 384], BF16, tag="qkts",
                                          name="qkts")
                    nc.scalar.activation(qkt_sb[:, 0:128],
                                         qkt_ps[:, 0:128], AF.Copy)
                    nc.scalar.activation(qkt_sb[:, 128:192],
                                         qkt_ps[:, 128:192], AF.Copy)
                    nc.scalar.activation(qkt_sb[:, 192:320],
                                         qkt_ps[:, 256:384], AF.Copy)
                    nc.scalar.activation(qkt_sb[:, 320:384],
                                         qkt_ps[:, 192:256], AF.Copy)
                    # only the FIRST matmul starts the bank's lazy-zero
                    # region (it spans all 128 partitions x whole bank);
                    # the second group's first write then sees its range
                    # fully pending and overwrites, later writes accumulate.
                    nc.tensor.matmul(
                        g_ps[0], qkt_sb[:, 0:128], qkt_sb[:, C:2 * C],
                        start=(g == 0), stop=(g == NT * NCHUNK - 1),
                        skip_group_check=True)
                    nc.tensor.matmul(
                        g_ps[1], qkt_sb[:, 128:192], qkt_sb[:, C:2 * C],
                        start=False, stop=(g == NT * NCHUNK - 1),
                        skip_group_check=True)

            prev = None
            for t in range(NT):
                r0 = t * TR
                xt = [xpool.tile([128, PR * W], BF16, tag="x0", name="x0"),
                      xpool.tile([64, PR * W], BF16, tag="x1", name="x1")]
                lo = r0 - 1
                hi = r0 + TR + 1
                dlo = max(lo, 0)
                dhi = min(hi, H)
                off = dlo - lo
                for ci, (cs, cp) in enumerate(((0, 128), (128, 64))):
                    if lo < 0:
                        nc.vector.memset(xt[ci][:, 0:W], 0.0)
                    if hi > H:
                        nc.vector.memset(xt[ci][:, (PR - 1) * W:PR * W], 0.0)
                    nc.gpsimd.dma_start(
                        out=xt[ci][:, off * W:(off + dhi - dlo) * W],
                        in_=x_d[cs:cs + cp, dlo:dhi, :],
                    )
                ft_a = fpool.tile([128, S], BF16, tag="fa", name="fa")
                nc.gpsimd.dma_start(out=ft_a[:], in_=f_d[0:128, r0:r0 + TR, :])
                ft_b = fpool.tile([128, S], BF16, tag="fb", name="fb")
                nc.gpsimd.dma_start(out=ft_b[0:64, :], in_=f_d[128:192, r0:r0 + TR, :])
                nc.gpsimd.dma_start(out=ft_b[64:128, :], in_=f_d[128:192, r0:r0 + TR, :])

                # ---- qkv matmul (5 blocks, 10 halo rows) + psum->sbuf ----
                sb = []
                ms = 0
                for bi, psz in enumerate(BLK):
                    q_sb = qkvpool.tile([psz, PR * PW], BF16, tag=f"sb{bi}",
                                        name=f"sb{bi}")
                    q3 = q_sb.rearrange("p (r w) -> p r w", w=PW)
                    sb.append(q3)
                    nc.vector.memset(q3[:, :, 1:2], 0.0)
                    nc.vector.memset(q3[:, :, 130:131], 0.0)
                    for c0, csz in ((0, 4), (4, 4), (8, 2)):
                        ps = mmpsum.tile([psz, 4 * W], F32, tag="mmps",
                                         name="mmps", bufs=3)
                        nc.tensor.matmul(
                            ps[:, :csz * W],
                            wq_sb[0][:, ms:ms + psz],
                            xt[0][:, c0 * W:(c0 + csz) * W],
                            start=True, stop=False)
                        nc.tensor.matmul(
                            ps[:, :csz * W],
                            wq_sb[1][:, ms:ms + psz],
                            xt[1][:, c0 * W:(c0 + csz) * W],
                            start=False, stop=True)
                        ps3 = ps[:, :csz * W].rearrange("p (r w) -> p r w", w=W)
                        # middle chunk of the 128-ch blocks drains via DVE:
                        # keeps the qkv phase matmul-paced instead of
                        # Scalar-copy-paced without flooding the DVE queue
                        if c0 == 4 and bi in (0, 2, 3):
                            nc.vector.tensor_copy(
                                q3[:, c0:c0 + csz, 2:2 + W], ps3)
                        else:
                            nc.scalar.activation(
                                q3[:, c0:c0 + csz, 2:2 + W], ps3, AF.Copy)
                    ms += psz

                # previous tile's tail goes after this tile's qkv matmuls
                if prev is not None:
                    emit_tail(*prev)

                # ---- q-hi/k-hi stencil on DVE (block B1 only) ----
                bi = 1
                psz = BLK[bi]
                st1 = stpool.tile([psz, S], BF16, tag="st1", name="st1")
                acc_ap = st1.rearrange("p (r w) -> p r w", w=W)
                dve_taps = [(1, -1, 0)] + [tp for tp in TAP_OFF if tp[0] != 1]
                for idx, (ti, dy, dx) in enumerate(dve_taps):
                    src = sb[bi][:, 1 + dy:1 + dy + TR, 2 + dx:2 + dx + W]
                    w_ap = taps_sb[bi][:, ti:ti + 1]
                    if idx == 0:
                        nc.vector.tensor_scalar_mul(acc_ap, src, w_ap)
                    else:
                        nc.vector.scalar_tensor_tensor(
                            acc_ap, src, w_ap, acc_ap,
                            op0=AL.mult, op1=AL.add)

                # qf/kf multiply for B1 (in place) + squares
                nc.vector.tensor_mul(st1[:], st1[:], ft_b[:])
                scr1 = scrpool.tile([128, S], BF16, tag="scr1", name="scr1")
                nc.scalar.activation(
                    scr1[:], st1[:], AF.Square,
                    accum_out=sq_sb[1][:, t:t + 1])

                prev = (t, st1, sb, ft_a, ft_b)

            emit_tail(*prev)

            # ================= final small-tensor phase =================
            kwq = [0]

            def keep_warm(lhs_tile, kcnt, mcnt):
                """Tiny matmul dependent on a just-produced small tile to
                keep the PE HAM window busy through the scalar phase."""
                ps = mmpsum.tile([128, 512], F32, tag="mmps", name="kw",
                                 bufs=3)
                ident = identf if lhs_tile.dtype == F32 else identb
                nc.tensor.matmul(ps[0:mcnt, 0:64], lhs_tile,
                                 ident[0:kcnt, 0:64], start=True, stop=True)
                kwq[0] += 1

            dmaq = [nc.sync, nc.gpsimd]

            rb = []
            for i in range(3):
                sq1 = finpool.tile([128, 1], F32, tag=f"sq1_{i}", name=f"sq1_{i}")
                nc.vector.tensor_reduce(
                    sq1[:], sq_sb[i][:], axis=mybir.AxisListType.X, op=AL.add)
                nc.vector.tensor_scalar_max(sq1[:], sq1[:], 1e-24)
                nq = finpool.tile([128, 1], F32, tag=f"nq_{i}", name=f"nq_{i}")
                nc.scalar.activation(nq[:], sq1[:], AF.Sqrt)
                r = finpool.tile([128, 1], F32, tag=f"rq_{i}", name=f"rq_{i}")
                nc.vector.reciprocal(r[:], nq[:])
                rb.append(r)
                keep_warm(r[:], 128, 1)

            G_sb = [finpool.tile([128, C], F32, tag="G0", name="G0"),
                    finpool.tile([64, C], F32, tag="G1", name="G1")]
            nc.vector.tensor_scalar_mul(G_sb[0][:], g_ps[0], rb[0][:])
            nc.vector.tensor_scalar_mul(G_sb[1][:], g_ps[1], rb[1][0:64, :])

            gt0_t = vpsum.tile([128, 512], F32, tag="vp3", name="gt0")
            gt1_t = vpsum.tile([64, 512], F32, tag="vp4", name="gt1")
            gt_ps = [gt0_t[:, 0:C], gt1_t[:, 0:C]]
            nc.tensor.matmul(gt_ps[0][:, 0:128], G_sb[0][:, 0:128], identf[:],
                             is_transpose=True, start=True, stop=True)
            nc.tensor.matmul(gt_ps[0][:, 128:192], G_sb[1][:, 0:128],
                             identf[0:64, 0:64], is_transpose=True,
                             start=True, stop=True)
            nc.tensor.matmul(gt_ps[1][:, 0:128], G_sb[0][:, 128:192], identf[:],
                             is_transpose=True, start=True, stop=True)
            nc.tensor.matmul(gt_ps[1][:, 128:192], G_sb[1][:, 128:192],
                             identf[0:64, 0:64], is_transpose=True,
                             start=True, stop=True)

            # per-partition rk * temperature vectors for gt rows (d)
            rkt_lo = finpool.tile([128, 1], F32, tag="rktl", name="rktl")
            nc.vector.tensor_mul(rkt_lo[:], rb[2][:], tlo_sb[:])
            rkh0 = finpool.tile([64, 1], F32, tag="rkh0", name="rkh0")
            nc.gpsimd.dma_start(out=rkh0[:], in_=rb[1][64:128, :])
            rkt_hi = finpool.tile([64, 1], F32, tag="rkth", name="rkth")
            nc.vector.tensor_mul(rkt_hi[:], rkh0[:], thi_sb[:])
            keep_warm(rkt_lo[:], 128, 1)

            gt_sb = [finpool.tile([128, C], F32, tag="gts0", name="gts0"),
                     finpool.tile([64, C], F32, tag="gts1", name="gts1")]
            nc.vector.tensor_scalar_mul(gt_sb[0][:], gt_ps[0], rkt_lo[:])
            nc.vector.tensor_scalar_mul(gt_sb[1][:], gt_ps[1], rkt_hi[:])
            keep_warm(gt_sb[0][:, 0:64], 128, 64)
            # ---- softmax without gathers: exp the full scaled Gt,
            # per-head denominators via block-ones matmul, division folded
            # into W_proj row-scaling, cross-head masking fused into the
            # attn-transpose copy ----
            e_lo = finpool.tile([128, C], F32, tag="elo", name="elo")
            nc.scalar.activation(e_lo[:], gt_sb[0][:], AF.Exp)
            e_hi = finpool.tile([64, C], F32, tag="ehi", name="ehi")
            nc.scalar.activation(e_hi[:], gt_sb[1][:], AF.Exp)
            # denominators: sums[h, c] = sum_{d in head h} e[d, c]
            sums_t = tpsum.tile([128, 512], F32, tag="qkt", name="sums")
            sums_ps = sums_t[0:8, 0:C]
            nc.tensor.matmul(sums_ps, ones_lo[:], e_lo[:],
                             start=True, stop=False)
            nc.tensor.matmul(sums_ps, ones_hi[:], e_hi[:],
                             start=False, stop=True)
            sums_sb = finpool.tile([8, C], F32, tag="sums", name="sums")
            nc.vector.tensor_copy(sums_sb[:], sums_ps)
            # attn^T (c on partitions) with cross-head masking in the copy
            aT_t = tpsum.tile([128, 512], F32, tag="qkt", name="aT")
            nc.tensor.matmul(aT_t[0:96, 0:128], e_lo[:, 0:96], identf[:],
                             start=True, stop=True, skip_group_check=True)
            nc.tensor.matmul(aT_t[0:96, 128:192], e_hi[:, 0:96],
                             identf[0:64, 0:64], start=True, stop=True,
                             skip_group_check=True)
            nc.tensor.matmul(aT_t[0:96, 256:384], e_lo[:, 96:192], identf[:],
                             start=True, stop=True, skip_group_check=True)
            nc.tensor.matmul(aT_t[0:96, 384:448], e_hi[:, 96:192],
                             identf[0:64, 0:64], start=True, stop=True,
                             skip_group_check=True)
            # transpose sums -> [96, 8] halves, pick own-head entry, recip
            sT_t = tpsum.tile([128, 512], F32, tag="qkt", name="sT")
            nc.tensor.matmul(sT_t[0:96, 0:8], sums_sb[:, 0:96],
                             identf[0:8, 0:8], start=True, stop=True,
                             skip_group_check=True)
            nc.tensor.matmul(sT_t[0:96, 8:16], sums_sb[:, 96:192],
                             identf[0:8, 0:8], start=True, stop=True,
                             skip_group_check=True)
            den = finpool.tile([96, 16], F32, tag="den", name="den")
            nc.vector.tensor_mul(den[:, 0:8], sT_t[0:96, 0:8], mc_lo[:])
            nc.vector.tensor_mul(den[:, 8:16], sT_t[0:96, 8:16], mc_hi[:])
            dsum = finpool.tile([96, 2], F32, tag="dsum", name="dsum")
            nc.vector.tensor_reduce(
                dsum[:, 0:1], den[:, 0:8], axis=mybir.AxisListType.X,
                op=AL.add)
            nc.vector.tensor_reduce(
                dsum[:, 1:2], den[:, 8:16], axis=mybir.AxisListType.X,
                op=AL.add)
            rsr = finpool.tile([96, 2], F32, tag="rsr", name="rsr")
            nc.vector.reciprocal(rsr[:], dsum[:])
            # scaled projection weights: wps[k] = wp[k] * (1/denominator)
            wps_sb = [finpool.tile([96, C], BF16, tag="wps0", name="wps0"),
                      finpool.tile([96, C], BF16, tag="wps1", name="wps1")]
            nc.vector.tensor_scalar_mul(wps_sb[0][:], wp_sb[0][:],
                                        rsr[:, 0:1])
            nc.vector.tensor_scalar_mul(wps_sb[1][:], wp_sb[1][:],
                                        rsr[:, 1:2])
            # masked attn^T to SBUF (bf16)
            attnT = [finpool.tile([96, C], BF16, tag="aT0", name="aT0"),
                     finpool.tile([96, C], BF16, tag="aT1", name="aT1")]
            nc.vector.tensor_mul(attnT[0][:], aT_t[0:96, 0:C], mT_lo[:])
            nc.vector.tensor_mul(attnT[1][:], aT_t[0:96, 256:256 + C],
                                 mT_hi[:])
            mt0_t = tpsum.tile([128, 512], F32, tag="qkt", name="mt0")
            mt1_t = tpsum.tile([128, 512], F32, tag="qkt", name="mt1")
            mt_ps = [mt0_t[:, 0:C], mt1_t[0:64, 0:C]]
            for mi, msl in enumerate((slice(0, 128), slice(128, 192))):
                for k in range(2):
                    nc.tensor.matmul(mt_ps[mi], attnT[k][:, msl], wps_sb[k][:],
                                     start=(k == 0), stop=(k == 1))
            # dense PE burst (~4us) to push the HAM clock-gate back to
            # 8/8 before the long out-matmul stream; results unused
            for wi in range(18):
                wps = mmpsum.tile([128, 512], F32, tag="mmps", name="wps",
                                  bufs=3)
                nc.tensor.matmul(wps[:], identb[:],
                                 v_sb[0][:, wi * 512:(wi + 1) * 512],
                                 start=True, stop=True)
            mt_sb = [finpool.tile([128, C], BF16, tag="mt_sb0", name="mt_sb0"),
                     finpool.tile([64, C], BF16, tag="mt_sb1", name="mt_sb1")]
            nc.vector.tensor_copy(mt_sb[0][:], mt_ps[0])
            nc.vector.tensor_copy(mt_sb[1][:], mt_ps[1])

            # out matmuls in groups of 3 columns-chunks per weight load:
            # same lhsT streams 3 chunks back-to-back (one LDWEIGHTS per 3
            # matmuls), 6 PSUM banks in flight (mmps x3 + qkt x2 + vp3)
            jlist = list(range(N // 512))
            gi = 0
            gsizes = [3] * 8 + [2] * 3 + [1] * 2
            while jlist:
                gs = gsizes.pop(0)
                grp = jlist[:gs]
                jlist = jlist[gs:]
                ps0 = {}
                ps1 = {}
                for gj, j in enumerate(grp):
                    ps0[j] = mmpsum.tile([128, 512], F32, tag="mmps",
                                         name="mmps", bufs=3)
                    if gj < 2:
                        ps1[j] = tpsum.tile([128, 512], F32, tag="qkt",
                                            name="ops")[0:64, :]
                    else:
                        ps1[j] = vpsum.tile([128, 512], F32, tag="vp3",
                                            name="ops")[0:64, :]
                for k in range(2):
                    for j in grp:
                        col = slice(j * 512, (j + 1) * 512)
                        nc.tensor.matmul(ps0[j][:], mt_sb[k][:, 0:128],
                                         v_sb[k][:, col],
                                         start=(k == 0), stop=(k == 1))
                for k in range(2):
                    for j in grp:
                        col = slice(j * 512, (j + 1) * 512)
                        nc.tensor.matmul(ps1[j][:], mt_sb[k][:, 128:192],
                                         v_sb[k][:, col],
                                         start=(k == 0), stop=(k == 1))
                gw = 512 * len(grp)
                gcol = slice(grp[0] * 512, grp[0] * 512 + gw)
                osb0 = outpool.tile([128, 1536], BF16, tag="osb0",
                                    name="osb0")
                osb1 = outpool.tile([64, 1536], BF16, tag="osb1",
                                    name="osb1")
                for gj, j in enumerate(grp):
                    gsl = slice(gj * 512, (gj + 1) * 512)
                    nc.scalar.activation(osb0[:, gsl], ps0[j][:], AF.Copy)
                    nc.vector.tensor_copy(osb1[0:64, gsl], ps1[j][:])
                dq = nc.sync if gi % 2 == 0 else nc.gpsimd
                dq.dma_start(out=out_d[0:128, gcol], in_=osb0[:, 0:gw])
                dq2 = nc.gpsimd if gi % 2 == 0 else nc.sync
                dq2.dma_start(out=out_d[128:192, gcol], in_=osb1[0:64, 0:gw])
                gi += 1
    nc.finalize()
    return nc


_NC_CACHE = {}


def _perm():
    return (list(range(0, 128)) + list(range(128, 192))
            + list(range(320, 384)) + list(range(192, 320))
            + list(range(384, 576)))


def _diag9(taps_blk, psz):
    d = np.zeros((psz, 9 * psz), np.float32)
    idx = np.arange(psz)
    for ti in range(9):
        d[:, ti * psz:(ti + 1) * psz][idx, idx] = taps_blk[:, ti]
    return d


def kernel(x, feature, W_qkv, W_dw, W_proj, temperature):
    import ml_dtypes
    b = x.shape[0]
    perm = _perm()
    wq_p = np.asarray(W_qkv, np.float32)[perm, :]
    wq = np.ascontiguousarray(wq_p.T).astype(ml_dtypes.bfloat16)
    taps = np.ascontiguousarray(
        np.asarray(W_dw, np.float32).reshape(O, 9)[perm, :])
    vd3 = _diag9(taps[384:512, :], 128)
    vd4 = _diag9(taps[512:576, :], 64)
    kd3 = _diag9(taps[256:384, :], 128)
    qd3 = _diag9(taps[0:128, :], 128)
    wp = np.ascontiguousarray(np.asarray(W_proj, np.float32).T).astype(
        ml_dtypes.bfloat16)
    temp = np.broadcast_to(
        np.asarray(temperature, np.float32).reshape(1, HEADS), (CH, HEADS))
    temp = np.ascontiguousarray(temp)
    tvec = np.repeat(np.asarray(temperature, np.float32).reshape(HEADS), CH)
    tlo = np.ascontiguousarray(tvec[0:128].reshape(128, 1))
    thi = np.ascontiguousarray(tvec[128:192].reshape(64, 1))
    hid = np.arange(C) // CH
    ones_bd = (hid[:, None] == np.arange(HEADS)[None, :]).astype(np.float32)
    maskT = (hid[:, None] == hid[None, :]).astype(np.float32)

    if "nc" not in _NC_CACHE:
        _NC_CACHE["nc"] = build_nc()
    nc = _NC_CACHE["nc"]

    in_maps = []
    for i in range(b):
        in_maps.append({
            "x": np.ascontiguousarray(np.asarray(x[i], np.float32)),
            "f": np.ascontiguousarray(np.asarray(feature[i], np.float32)),
            "wq": wq, "taps": taps,
            "vdiag3": vd3.astype(ml_dtypes.bfloat16),
            "vdiag4": vd4.astype(ml_dtypes.bfloat16),
            "kdiag3": kd3.astype(ml_dtypes.bfloat16),
            "qdiag3": qd3.astype(ml_dtypes.bfloat16),
            "wp": wp, "temp": temp, "tlo": tlo, "thi": thi,
            "ones_lo": np.ascontiguousarray(ones_bd[0:128]),
            "ones_hi": np.ascontiguousarray(ones_bd[128:192]),
            "maskT_lo": np.ascontiguousarray(maskT[0:96]).astype(
                ml_dtypes.bfloat16),
            "maskT_hi": np.ascontiguousarray(maskT[96:192]).astype(
                ml_dtypes.bfloat16),
            "maskc_lo": np.ascontiguousarray(ones_bd[0:96]),
            "maskc_hi": np.ascontiguousarray(ones_bd[96:192]),
            "identb": np.eye(128, dtype=np.float32).astype(ml_dtypes.bfloat16),
            "identf": np.eye(128, dtype=np.float32),
        })
    res = run_bass_kernel_spmd(nc, in_maps, list(range(b)))
    outs = [np.asarray(r["out"], np.float32).reshape(C, H, W)
            for r in res.results]
    return np.stack(outs, axis=0)
